# revision 1
# baseline (speedup 1.0000x reference)
"""BarrierNet forward pass on 8 Trainium2 NeuronCores (pure data parallel).

Network (per sample, batch 8192 sharded 1024/core):
  x[5] -> 1024 -> 1024 -> {512, 512} -> {512, 512} -> two 2-wide heads
  followed by a closed-form single-constraint QP projection (dCBF barrier).

Layout strategy per core:
  - MLP runs feature-major: activations stored as [feat, batch] tiles so every
    layer is out_T[n] += W[k,n].T @ act_T[k] with K on partitions; batch free
    dim = 512 per matmul (f32r, effectively full-rate near-fp32).
  - Each 512-sample batch tile runs the full pipeline (L1..heads..epilogue) so
    the tile-0 epilogue (DVE/ACT) overlaps tile-1 matmuls on the PE.
  - Head rows land in [32, 512] staging tiles; one DVE 32x32 stream-transpose
    per tile flips them to batch-on-partition (sample j=32b+c at [c, 32b+row])
    keeping the PE free of transpose work (no HAM re-throttle).
  - The QP/barrier epilogue runs on [32, 16] strided views; all DVE/ACT
    elementwise ops.
  - Host does the cheap reshapes; all FLOPs run on device.
"""

import numpy as np

import concourse.bass as bass
import concourse.tile as tile
from concourse import bacc, mybir
from concourse.bass_utils import run_bass_kernel_spmd

N_CORES = 8
B_FULL = 8192
BC = B_FULL // N_CORES      # batch per core
BT = 512                    # batch tile (matmul moving free dim)
NBT = BC // BT              # batch tiles per core
GPB = BT // 32              # 32-sample groups per batch tile (16)

D1, D2, D3, D4 = 1024, 1024, 512, 512
L1C, L2C, OBS_X, OBS_Y, RADIUS = 3.0, 3.0, 0.0, 7.0, 4.0

# "f32r": full-rate matmul, near-fp32; "f32": exact, 4x slower; "bf16"
MM_MODE = "f32r"

F32 = mybir.dt.float32
AF = mybir.ActivationFunctionType
AL = mybir.AluOpType


def _mm_dt():
    return {"f32r": mybir.dt.float32r, "f32": F32, "bf16": mybir.dt.bfloat16}[MM_MODE]


def _act_np_dt():
    import ml_dtypes
    return np.dtype(ml_dtypes.bfloat16) if MM_MODE == "bf16" else np.float32


def build_program(consts):
    """Build the SPMD Bass program.
    consts = (mean[5], std[5], ml[2], sl[2], b51[2], b52[2])."""
    mean, std, ml, sl, b51v, b52v = consts
    act_dt = _mm_dt()

    nc = bacc.Bacc("TRN2", target_bir_lowering=False, debug=False,
                   num_devices=N_CORES)

    def din(name, shape, dt=None):
        if dt is None:
            dt = act_dt
        return nc.dram_tensor(name, shape, dt, kind="ExternalInput").ap()

    xT_d = din("xT", [5, BC])
    Xep_d = din("Xep", [32, NBT * GPB * 5], F32)
    W1_d = din("W1", [5, D1])
    W2_d = din("W2", [D1, D2])
    W31_d = din("W31", [D2, D3])
    W32_d = din("W32", [D2, D3])
    W41_d = din("W41", [D3, D4])
    W42_d = din("W42", [D3, D4])
    W51_d = din("W51p", [128, (D4 // 128) * 2])
    W52_d = din("W52p", [128, (D4 // 128) * 2])
    b1_d = din("b1p", [128, D1 // 128], F32)
    b2_d = din("b2p", [128, D2 // 128], F32)
    b31_d = din("b31p", [128, D3 // 128], F32)
    b32_d = din("b32p", [128, D3 // 128], F32)
    b41_d = din("b41p", [128, D4 // 128], F32)
    b42_d = din("b42p", [128, D4 // 128], F32)
    b51_d = din("b51p", [2, 1], F32)
    b52_d = din("b52p", [2, 1], F32)
    out_d = nc.dram_tensor("out", [32, NBT * GPB * 2], F32,
                           kind="ExternalOutput").ap()

    K2, K3, K4, K5 = D1 // 128, D2 // 128, D3 // 128, D4 // 128
    N1, N2, N3, N4 = D1 // 128, D2 // 128, D3 // 128, D4 // 128

    with tile.TileContext(nc) as tc:
        with (
            tc.tile_pool(name="wpool", bufs=1) as wp,
            tc.tile_pool(name="acts", bufs=32) as ap_,
            tc.tile_pool(name="misc", bufs=1) as mp,
            tc.tile_pool(name="ep", bufs=1) as ep,
            tc.tile_pool(name="pmm", bufs=6, space="PSUM") as pmm,
            tc.tile_pool(name="phead", bufs=2, space="PSUM") as phd,
        ):
            # ---- input/weight loads -------------------------------------
            # sync ring: L1 inputs first, then all weight matrices in order
            xT = mp.tile([5, BC], act_dt, tag="xT", name="xT_t")
            nc.sync.dma_start(out=xT, in_=xT_d)
            w1 = wp.tile([5, D1], act_dt, tag="w1", name="w1_t")
            nc.sync.dma_start(out=w1, in_=W1_d)

            # sync ring: weight matrices in need-order
            def chunked_w(dram, rows, cols, nm, engine):
                sl_ = []
                for k in range(rows // 128):
                    t = wp.tile([128, cols], act_dt, tag=f"{nm}{k}",
                                name=f"{nm}{k}_t")
                    engine.dma_start(out=t, in_=dram[k * 128:(k + 1) * 128, :])
                    sl_.append(t)
                return sl_

            w2 = chunked_w(W2_d, D1, D2, "w2", nc.sync)
            w31 = chunked_w(W31_d, D2, D3, "w31", nc.sync)
            w32 = chunked_w(W32_d, D2, D3, "w32", nc.sync)
            w41 = chunked_w(W41_d, D3, D4, "w41", nc.sync)
            w42 = chunked_w(W42_d, D3, D4, "w42", nc.sync)

            # gpsimd ring: W31/W32 + the small late tensors
            def gp_load(dram, shape, tg, dt=F32):
                t = mp.tile(shape, dt, tag=tg, name=f"{tg}_t")
                nc.gpsimd.dma_start(out=t, in_=dram)
                return t

            b1 = gp_load(b1_d, [128, N1], "b1")
            Xep = gp_load(Xep_d, [32, NBT * GPB * 5], "Xep")
            b2 = gp_load(b2_d, [128, N2], "b2")
            b31 = gp_load(b31_d, [128, N3], "b31")
            b32 = gp_load(b32_d, [128, N3], "b32")
            b41 = gp_load(b41_d, [128, N4], "b41")
            b42 = gp_load(b42_d, [128, N4], "b42")
            w51 = gp_load(W51_d, [128, K5 * 2], "w51", act_dt)
            w52 = gp_load(W52_d, [128, K5 * 2], "w52", act_dt)
            b51 = gp_load(b51_d, [2, 1], "b51")
            b52 = gp_load(b52_d, [2, 1], "b52")

            OUT = mp.tile([32, NBT * GPB * 2], F32, tag="OUT", name="OUT_t")

            _cbias_cache = {}

            def cbias(val, parts):
                val = float(val)
                if val not in _cbias_cache:
                    t = ep.tile([128, 1], F32, tag=f"cb{len(_cbias_cache)}",
                                name=f"cb{len(_cbias_cache)}")
                    nc.vector.memset(t, val)
                    _cbias_cache[val] = t
                return _cbias_cache[val][0:parts, :]

            def eact(out, in_, func, bias=0.0, scale=1.0):
                if isinstance(bias, float) and func not in (AF.Copy,):
                    bias = cbias(bias, in_.shape[0])
                nc.scalar.activation(out, in_, func, bias=bias, scale=scale)

            def relu_bias(t, ps, bias_col, n):
                if n % 2 == 0:
                    nc.vector.tensor_scalar(t, ps, bias_col, 0.0,
                                            AL.add, AL.max)
                else:
                    nc.scalar.activation(t, ps, AF.Relu, bias=bias_col)

            HPI = float(np.pi / 2)
            PI = float(np.pi)

            def epilogue_pre(bt):
                """x-only QP/barrier quantities for batch tile bt (no head
                dependence) — runs on DVE/ACT while the PE is still in the
                dense layers."""
                Xv = Xep[:, bt * GPB * 5:(bt + 1) * GPB * 5] \
                    .rearrange("p (f j) -> p f j", j=5)

                def T(nm):
                    return ep.tile([32, GPB], F32, tag=nm, bufs=NBT,
                                   name=f"{nm}_b{bt}")

                def emul(o, a, b):
                    nc.vector.tensor_mul(o, a, b)

                def eadd(o, a, b):
                    nc.vector.tensor_add(o, a, b)

                def stt(o, a, s, op0, b, op1):
                    nc.vector.scalar_tensor_tensor(o, a, float(s), b, op0, op1)

                t1r, w1r = Xv[:, :, 0], Xv[:, :, 1]
                t2r, w2r = Xv[:, :, 2], Xv[:, :, 3]

                if float(std[0]) == 1.0 and float(mean[0]) == 0.0:
                    t1m = t1r
                else:
                    t1m = T("t1m"); eact(t1m, t1r, AF.Copy, bias=float(mean[0]), scale=float(std[0]))
                if float(std[2]) == 1.0 and float(mean[2]) == 0.0:
                    t2m = t2r
                else:
                    t2m = T("t2m"); eact(t2m, t2r, AF.Copy, bias=float(mean[2]), scale=float(std[2]))

                def sincos(theta, nm):
                    ws = T(nm + "_ws"); nc.vector.add_range_wrap(ws, theta, 0.0, PI, 2 * PI)
                    s = T(nm + "_s"); eact(s, ws, AF.Sin)
                    wc = T(nm + "_wc"); nc.vector.add_range_wrap(wc, theta, HPI, PI, 2 * PI)
                    c = T(nm + "_c"); eact(c, wc, AF.Sin)
                    return s, c

                s1, c1 = sincos(t1m, "t1")
                s2, c2 = sincos(t2m, "t2")

                if float(std[1]) == 1.0 and float(mean[1]) == 0.0:
                    w1v = w1r
                else:
                    w1v = T("w1v"); eact(w1v, w1r, AF.Copy, bias=float(mean[1]), scale=float(std[1]))
                if float(std[3]) == 1.0 and float(mean[3]) == 0.0:
                    w2v = w2r
                else:
                    w2v = T("w2v"); eact(w2v, w2r, AF.Copy, bias=float(mean[3]), scale=float(std[3]))

                pxu = T("pxu"); eadd(pxu, c1, c2)
                px = T("px"); eact(px, pxu, AF.Copy, bias=-OBS_X, scale=L1C)
                pyu = T("pyu"); eadd(pyu, s1, s2)
                py = T("py"); eact(py, pyu, AF.Copy, bias=-OBS_Y, scale=L1C)

                a1 = T("a1"); emul(a1, s1, w1v)
                a2 = T("a2"); emul(a2, s2, w2v)
                vxn = T("vxn"); eadd(vxn, a1, a2)          # = -vx/3
                bb1 = T("bb1"); emul(bb1, c1, w1v)
                bb2 = T("bb2"); emul(bb2, c2, w2v)
                vyu = T("vyu"); eadd(vyu, bb1, bb2)
                vy = T("vy"); eact(vy, vyu, AF.Copy, scale=3.0)

                q1 = T("q1"); emul(q1, px, vxn)
                q2 = T("q2"); emul(q2, py, vy)
                bdot2 = T("bdot2"); stt(bdot2, q1, -3.0, AL.mult, q2, AL.add)

                w1sq = T("w1sq"); emul(w1sq, w1v, w1v)
                w2sq = T("w2sq"); emul(w2sq, w2v, w2v)
                cw1 = T("cw1"); emul(cw1, c1, w1sq)
                cw2 = T("cw2"); emul(cw2, c2, w2sq)
                cw = T("cw"); eadd(cw, cw1, cw2)
                sw1 = T("sw1"); emul(sw1, s1, w1sq)
                sw2 = T("sw2"); emul(sw2, s2, w2sq)
                sw = T("sw"); eadd(sw, sw1, sw2)
                t1x = T("t1x"); emul(t1x, px, cw)
                t2y = T("t2y"); emul(t2y, py, sw)
                txy = T("txy"); eadd(txy, t1x, t2y)
                vv1 = T("vv1"); emul(vv1, vxn, vxn)
                vv2 = T("vv2"); emul(vv2, vy, vy)
                vv = T("vv"); stt(vv, vv1, 9.0, AL.mult, vv2, AL.add)
                Lhalf = T("Lhalf"); stt(Lhalf, txy, -3.0, AL.mult, vv, AL.add)

                g1a = T("g1a"); emul(g1a, px, s1)
                g1b = T("g1b"); emul(g1b, py, c1)
                g2a = T("g2a"); emul(g2a, px, s2)
                g2b = T("g2b"); emul(g2b, py, c2)
                G12 = ep.tile([32, GPB * 2], F32, tag="G12", bufs=NBT,
                              name=f"G12_b{bt}")
                G12v = G12.rearrange("p (f q) -> p f q", q=2)
                G1h, G2h = G12v[:, :, 0], G12v[:, :, 1]
                stt(G1h, g1b, -1.0, AL.mult, g1a, AL.add)  # G1/6
                stt(G2h, g2b, -1.0, AL.mult, g2a, AL.add)  # G2/6

                pxsq = T("pxsq"); emul(pxsq, px, px)
                pysq = T("pysq"); emul(pysq, py, py)
                bar = T("bar"); stt(bar, pxsq, -RADIUS * RADIUS, AL.add, pysq, AL.add)

                d1 = T("d1"); emul(d1, G1h, G1h)
                d2 = T("d2"); emul(d2, G2h, G2h)
                den36 = T("den36"); stt(den36, d1, 1e-12 / 36.0, AL.add, d2, AL.add)
                nrec = T("nrec"); nc.vector.reciprocal(nrec, den36)

                return dict(bdot2=bdot2, bar=bar, Lhalf=Lhalf,
                            G1h=G1h, G2h=G2h, G12=G12, nrec=nrec)

            def epilogue_post(bt, vta, vtb, pre):
                """Head-dependent tail of the QP for batch tile bt."""
                Yva = vta.rearrange("p (f q) -> p f q", q=32)
                Yvb = vtb.rearrange("p (f q) -> p f q", q=32)
                P1, P2 = Yva[:, :, 0], Yva[:, :, 1]
                sg1, sg2 = Yvb[:, :, 0], Yvb[:, :, 1]
                OUTv = OUT[:, bt * GPB * 2:(bt + 1) * GPB * 2] \
                    .rearrange("p (f i) -> p f i", i=2)

                def T(nm):
                    return ep.tile([32, GPB], F32, tag=nm, bufs=NBT,
                                   name=f"{nm}_b{bt}")

                def emul(o, a, b):
                    nc.vector.tensor_mul(o, a, b)

                def eadd(o, a, b):
                    nc.vector.tensor_add(o, a, b)

                def stt(o, a, s, op0, b, op1):
                    nc.vector.scalar_tensor_tensor(o, a, float(s), b, op0, op1)

                bdot2, bar, Lhalf = pre["bdot2"], pre["bar"], pre["Lhalf"]
                G1h, G2h, nrec = pre["G1h"], pre["G2h"], pre["nrec"]
                G12 = pre["G12"]
                P12 = Yva[:, :, 0:2]  # [32, GPB, 2]

                ssum = T("ssum"); eadd(ssum, sg1, sg2)
                sprod = T("sprod"); emul(sprod, sg1, sg2)
                hb = T("hb"); emul(hb, ssum, bdot2)
                hc = T("hc"); emul(hc, sprod, bar)

                r12 = ep.tile([32, GPB * 2], F32, tag="r12", bufs=NBT,
                              name=f"r12_b{bt}")
                r12v = r12.rearrange("p (f q) -> p f q", q=2)
                nc.vector.tensor_mul(r12v, G12.rearrange("p (f q) -> p f q", q=2), P12)
                rs = T("rs"); eadd(rs, r12v[:, :, 0], r12v[:, :, 1])
                va2 = T("va2"); stt(va2, hc, 8.0, AL.mult, Lhalf, AL.add)
                va = T("va"); stt(va, hb, 4.0, AL.mult, va2, AL.add)     # h/2
                vb = T("vb"); stt(vb, rs, 3.0, AL.mult, va, AL.add)      # viol=-2vb

                vr = T("vr")
                nc.vector.tensor_scalar(vr, vb, -1.0, 0.0, AL.mult, AL.max)
                lam18 = T("lam18"); emul(lam18, vr, nrec)

                lam18b = bass.AP(tensor=lam18.tensor, offset=lam18.offset,
                                 ap=list(lam18.ap) + [[0, 2]])
                lg12 = ep.tile([32, GPB * 2], F32, tag="lg12", bufs=NBT,
                               name=f"lg12_b{bt}")
                lg12v = lg12.rearrange("p (f q) -> p f q", q=2)
                nc.vector.tensor_mul(
                    lg12v, lam18b, G12.rearrange("p (f q) -> p f q", q=2))
                if (float(sl[0]) == 1.0 and float(sl[1]) == 1.0
                        and float(ml[0]) == 0.0 and float(ml[1]) == 0.0):
                    # out = -(lg12/3 + P12): one DVE op straight into OUT
                    stt(OUTv[:, :, 0:2], lg12v, -1.0 / 3.0, AL.mult, P12,
                        AL.subtract)
                else:
                    u12n = ep.tile([32, GPB * 2], F32, tag="u12n", bufs=NBT,
                                   name=f"u12n_b{bt}")
                    u12v = u12n.rearrange("p (f q) -> p f q", q=2)
                    stt(u12v, lg12v, 1.0 / 3.0, AL.mult, P12, AL.add)
                    eact(OUTv[:, :, 0], u12v[:, :, 0], AF.Copy,
                         bias=-float(ml[0]) / float(sl[0]),
                         scale=-1.0 / float(sl[0]))
                    eact(OUTv[:, :, 1], u12v[:, :, 1], AF.Copy,
                         bias=-float(ml[1]) / float(sl[1]),
                         scale=-1.0 / float(sl[1]))

            def layer1(bt):
                """L1 for one batch tile (runs up front for both tiles)."""
                sfx = f"b{bt}"
                x1 = []
                rhs1 = xT[:, bt * BT:(bt + 1) * BT]
                for n in range(N1):
                    ps = pmm.tile([128, BT], F32, tag="pm", name=f"ps1_{n}{sfx}")
                    nc.tensor.matmul(ps, w1[:, n * 128:(n + 1) * 128], rhs1,
                                     start=True, stop=True)
                    t = ap_.tile([128, BT], act_dt, tag="act",
                                 name=f"x1_{n}{sfx}")
                    relu_bias(t, ps, b1[:, n:n + 1], n)
                    x1.append(t)
                return x1

            def batch_tile_pipeline(bt, x1):
                """Layers 2+ for one 512-sample batch tile."""
                sfx = f"b{bt}"

                def act_tile(nm):
                    return ap_.tile([128, BT], act_dt, tag="act", name=nm)

                def dense(nm, x_in, ws, bias, n_out, k_in):
                    out = []
                    for n in range(n_out):
                        ps = pmm.tile([128, BT], F32, tag="pm",
                                      name=f"ps{nm}_{n}{sfx}")
                        for k in range(k_in):
                            nc.tensor.matmul(
                                ps, ws[k][:, n * 128:(n + 1) * 128], x_in[k],
                                start=(k == 0), stop=(k == k_in - 1))
                        t = act_tile(f"{nm}_{n}{sfx}")
                        relu_bias(t, ps, bias[:, n:n + 1], n)
                        out.append(t)
                    return out

                x2 = dense("x2", x1, w2, b2, N2, K2)
                x31 = dense("x31", x2, w31, b31, N3, K3)
                x32 = dense("x32", x2, w32, b32, N3, K3)
                x41 = dense("x41", x31, w41, b41, N4, K4)
                x42 = dense("x42", x32, w42, b42, N4, K4)

                pre = epilogue_pre(bt)

                # heads: rows 0:2 of [32, BT] staging tiles
                x5a = mp.tile([32, BT], F32, tag="x5a", bufs=2, name=f"x5a{sfx}")
                x5b = mp.tile([32, BT], F32, tag="x5b", bufs=2, name=f"x5b{sfx}")
                nc.vector.memset(x5a, 0.0)
                nc.vector.memset(x5b, 0.0)
                ph1 = phd.tile([2, BT], F32, tag="ph", name=f"ph1{sfx}")
                for k in range(K5):
                    nc.tensor.matmul(ph1, w51[:, k * 2:(k + 1) * 2], x41[k],
                                     start=(k == 0), stop=(k == K5 - 1))
                nc.scalar.activation(x5a[0:2, :], ph1, AF.Identity, bias=b51)
                ph2 = phd.tile([2, BT], F32, tag="ph", name=f"ph2{sfx}")
                for k in range(K5):
                    nc.tensor.matmul(ph2, w52[:, k * 2:(k + 1) * 2], x42[k],
                                     start=(k == 0), stop=(k == K5 - 1))
                nc.scalar.activation(x5b[0:2, :], ph2, AF.Sigmoid, bias=b52)

                # DVE 32x32 stream transpose: sample j=32b+c -> [c, 32b+row]
                vta = mp.tile([32, BT], F32, tag="vta", bufs=2, name=f"vta{sfx}")
                vtb = mp.tile([32, BT], F32, tag="vtb", bufs=2, name=f"vtb{sfx}")
                nc.vector.transpose(vta, x5a)
                nc.vector.transpose(vtb, x5b)

                epilogue_post(bt, vta, vtb, pre)
                nc.sync.dma_start(
                    out=out_d[:, bt * GPB * 2:(bt + 1) * GPB * 2],
                    in_=OUT[:, bt * GPB * 2:(bt + 1) * GPB * 2])

            x1_all = [layer1(bt) for bt in range(NBT)]
            for bt in range(NBT):
                batch_tile_pipeline(bt, x1_all[bt])

    nc.compile()
    return nc


def prep_inputs(x, W1, b1, W2, b2, W31, b31, W32, b32,
                W41, b41, W42, b42, W51, b51, W52, b52):
    """Host-side reshapes -> per-core in_maps."""
    adt = _act_np_dt()
    f32 = np.float32

    def conv(a):
        return np.ascontiguousarray(np.asarray(a, f32).astype(adt))

    shared = {
        "W1": conv(W1), "W2": conv(W2),
        "W31": conv(W31), "W32": conv(W32),
        "W41": conv(W41), "W42": conv(W42),
        "W51p": conv(np.asarray(W51, f32).reshape(4, 128, 2)
                     .transpose(1, 0, 2).reshape(128, 8)),
        "W52p": conv(np.asarray(W52, f32).reshape(4, 128, 2)
                     .transpose(1, 0, 2).reshape(128, 8)),
        "b1p": np.ascontiguousarray(np.asarray(b1, f32).reshape(-1, 128).T),
        "b2p": np.ascontiguousarray(np.asarray(b2, f32).reshape(-1, 128).T),
        "b31p": np.ascontiguousarray(np.asarray(b31, f32).reshape(-1, 128).T),
        "b32p": np.ascontiguousarray(np.asarray(b32, f32).reshape(-1, 128).T),
        "b41p": np.ascontiguousarray(np.asarray(b41, f32).reshape(-1, 128).T),
        "b42p": np.ascontiguousarray(np.asarray(b42, f32).reshape(-1, 128).T),
        "b51p": np.asarray(b51, f32).reshape(2, 1).copy(),
        "b52p": np.asarray(b52, f32).reshape(2, 1).copy(),
    }
    x = np.asarray(x, f32)
    in_maps = []
    for c in range(N_CORES):
        xc = x[c * BC:(c + 1) * BC]
        m = dict(shared)
        m["xT"] = np.ascontiguousarray(xc.T.astype(adt))
        m["Xep"] = np.ascontiguousarray(
            xc.reshape(BC // 32, 32, 5).transpose(1, 0, 2)
            .reshape(32, (BC // 32) * 5))
        in_maps.append(m)
    return in_maps


def unpack_output(results):
    outs = []
    for c in range(N_CORES):
        o = results[c]["out"]  # [32, (BC//32)*2]
        outs.append(o.reshape(32, BC // 32, 2).transpose(1, 0, 2).reshape(BC, 2))
    return np.ascontiguousarray(np.concatenate(outs, axis=0), dtype=np.float32)


_PROG_CACHE = {}


def get_program(consts_key):
    if consts_key not in _PROG_CACHE:
        _PROG_CACHE[consts_key] = build_program(consts_key)
    return _PROG_CACHE[consts_key]


def kernel(x, sgn, mean, std, mean_label, std_label,
           W1, b1, W2, b2, W31, b31, W32, b32,
           W41, b41, W42, b42, W51, b51, W52, b52,
           _trace=False, _tmpdir=None):
    assert int(np.asarray(sgn)) == 1
    consts = (
        tuple(float(v) for v in np.asarray(mean, np.float32)),
        tuple(float(v) for v in np.asarray(std, np.float32)),
        tuple(float(v) for v in np.asarray(mean_label, np.float32)),
        tuple(float(v) for v in np.asarray(std_label, np.float32)),
        tuple(float(v) for v in np.asarray(b51, np.float32)),
        tuple(float(v) for v in np.asarray(b52, np.float32)),
    )
    nc = get_program(consts)
    in_maps = prep_inputs(x, W1, b1, W2, b2, W31, b31, W32, b32,
                          W41, b41, W42, b42, W51, b51, W52, b52)
    res = run_bass_kernel_spmd(nc, in_maps, core_ids=list(range(N_CORES)),
                               trace=_trace, tmpdir=_tmpdir)
    out = unpack_output(res.results)
    kernel.last_result = res
    return out



# revision 8
# speedup vs baseline: 1.6370x; 1.6370x over previous
"""BarrierNet forward pass on 8 Trainium2 NeuronCores (pure data parallel).

Network (per sample, batch 8192 sharded 1024/core):
  x[5] -> 1024 -> 1024 -> {512, 512} -> {512, 512} -> two 2-wide heads
  followed by a closed-form single-constraint QP projection (dCBF barrier).

v2: fp8(e4m3) DoubleRow matmuls for L2..L4 + heads (~1.44x PE throughput
vs f32r at FD=512). Precision plan (end-to-end rel err ~1.6e-3 in numpy
simulation, gate 2e-2):
  - Weights quantized per-tensor with power-of-2 scales chosen so each
    layer's PSUM comes out already in the next layer's storage scale:
    the PSUM->SBUF step is a single add-bias/relu/cast-fp8 instruction
    (Vector tensor_scalar or Scalar activation), no rescale pass.
  - That forces near-unity weight scales; fp8 subnormal storage of the
    uniform-init weights costs ~3.1% rms vs 2.7% at full range (the PE
    widens fp8 to e6m3 internally, honoring subnormals).
  - L1 stays f32r (K=5) with alpha1 folded into W1 host-side; activations
    stored with cascade scales alpha_l, calibrated from a host fp32
    forward of the actual batch (margin 192/240).
  - Heads run DoubleRow too (weights padded to 32 cols); the un-scaling
    1/(beta5*alpha4) enters via ACT scale APs, so the compiled program
    has no data-dependent immediates.
Layout strategy per core (as v1): feature-major [feat, batch] tiles,
BT=512 batch tiles, DVE 32x32 stream-transpose for the heads, QP/barrier
epilogue on [32, 16] strided views split across Vector/GpSimd/Scalar.
"""

import numpy as np

import concourse.bass as bass
import concourse.tile as tile
from concourse import bacc, mybir
from concourse.bass_utils import run_bass_kernel_spmd

N_CORES = 8
B_FULL = 8192
BC = B_FULL // N_CORES      # batch per core
BT = 512                    # batch tile (matmul moving free dim)
NBT = BC // BT              # batch tiles per core
GPB = BT // 32              # 32-sample groups per batch tile (16)

D1, D2, D3, D4 = 1024, 1024, 512, 512
KP2, KP3, KP4, KP5 = D1 // 256, D2 // 256, D3 // 256, D4 // 256  # k-pair counts
N1, N2, N3, N4 = D1 // 128, D2 // 128, D3 // 128, D4 // 128      # out chunks
L1C, L2C, OBS_X, OBS_Y, RADIUS = 3.0, 3.0, 0.0, 7.0, 4.0

MARGIN = 192.0              # fp8 activation headroom (max normal 240)

F32 = mybir.dt.float32
F32R = mybir.dt.float32r
FP8 = mybir.dt.float8e4
AF = mybir.ActivationFunctionType
AL = mybir.AluOpType
DR = mybir.MatmulPerfMode.DoubleRow

# bias_pack column offsets per layer
BOF = {"l1": 0, "l2": 8, "l31": 16, "l32": 20, "l41": 24, "l42": 28}


def build_program(consts):
    """Build the SPMD Bass program.
    consts = (mean[5], std[5], ml[2], sl[2])."""
    mean, std, ml, sl = consts

    nc = bacc.Bacc("TRN2", target_bir_lowering=False, debug=False,
                   num_devices=N_CORES)

    def din(name, shape, dt):
        return nc.dram_tensor(name, shape, dt, kind="ExternalInput").ap()

    xT_d = din("xT", [5, BC], F32R)
    W1_d = din("W1s", [5, D1], F32R)
    W2_d = din("W2p", [128, KP2 * 2 * D2], FP8)
    W31_d = din("W31p", [128, KP3 * 2 * D3], FP8)
    W32_d = din("W32p", [128, KP3 * 2 * D3], FP8)
    W41_d = din("W41p", [128, KP4 * 2 * D4], FP8)
    W42_d = din("W42p", [128, KP4 * 2 * D4], FP8)
    W5_d = din("W5p", [128, 2 * KP5 * 2 * 32], FP8)
    Xep_d = din("Xep", [32, NBT * GPB * 5], F32)
    bias_d = din("biasp", [128, 32], F32)
    hb_d = din("hbp", [2, 4], F32)   # cols: b51, b52, 1/(b51sc), 1/(b52sc)
    out_d = nc.dram_tensor("out", [32, NBT * GPB * 2], F32,
                           kind="ExternalOutput").ap()

    with tile.TileContext(nc) as tc:
        with (
            tc.tile_pool(name="wpool", bufs=1) as wp,
            tc.tile_pool(name="acts", bufs=16) as ap_,
            tc.tile_pool(name="misc", bufs=1) as mp,
            tc.tile_pool(name="ep", bufs=1) as ep,
            tc.tile_pool(name="pmm", bufs=6, space="PSUM") as pmm,
            tc.tile_pool(name="phead", bufs=2, space="PSUM") as phd,
        ):
            # ---- input/weight loads -------------------------------------
            xT = mp.tile([5, BC], F32R, tag="xT", name="xT_t")
            nc.sync.dma_start(out=xT, in_=xT_d)
            w1 = wp.tile([5, D1], F32R, tag="w1", name="w1_t")
            nc.sync.dma_start(out=w1, in_=W1_d)

            def sync_load(dram, shape, tg, dt=FP8):
                t = wp.tile(shape, dt, tag=tg, name=f"{tg}_t")
                nc.sync.dma_start(out=t, in_=dram)
                return t

            w2 = sync_load(W2_d, [128, KP2 * 2 * D2], "w2")
            w31 = sync_load(W31_d, [128, KP3 * 2 * D3], "w31")
            w32 = sync_load(W32_d, [128, KP3 * 2 * D3], "w32")
            w41 = sync_load(W41_d, [128, KP4 * 2 * D4], "w41")
            w42 = sync_load(W42_d, [128, KP4 * 2 * D4], "w42")

            def gp_load(dram, shape, tg, dt=F32):
                t = mp.tile(shape, dt, tag=tg, name=f"{tg}_t")
                nc.gpsimd.dma_start(out=t, in_=dram)
                return t

            Xep = gp_load(Xep_d, [32, NBT * GPB * 5], "Xep")
            biasp = gp_load(bias_d, [128, 32], "biasp")
            w5 = gp_load(W5_d, [128, 2 * KP5 * 2 * 32], "w5", FP8)
            hb = gp_load(hb_d, [2, 4], "hb")

            OUT = mp.tile([32, NBT * GPB * 2], F32, tag="OUT", name="OUT_t")

            # weight views: [128, pairs, plane, N]
            w2v = w2.rearrange("p (t i n) -> p t i n", t=KP2, i=2)
            w31v = w31.rearrange("p (t i n) -> p t i n", t=KP3, i=2)
            w32v = w32.rearrange("p (t i n) -> p t i n", t=KP3, i=2)
            w41v = w41.rearrange("p (t i n) -> p t i n", t=KP4, i=2)
            w42v = w42.rearrange("p (t i n) -> p t i n", t=KP4, i=2)
            w5v = w5.rearrange("p (h t i c) -> p h t i c", h=2, t=KP5, i=2)

            _cbias_cache = {}

            def cbias(val, parts):
                val = float(val)
                if val not in _cbias_cache:
                    t = ep.tile([128, 1], F32, tag=f"cb{len(_cbias_cache)}",
                                name=f"cb{len(_cbias_cache)}")
                    nc.vector.memset(t, val)
                    _cbias_cache[val] = t
                return _cbias_cache[val][0:parts, :]

            def eact(out, in_, func, bias=0.0, scale=1.0):
                if isinstance(bias, float) and func not in (AF.Copy,):
                    bias = cbias(bias, in_.shape[0])
                nc.scalar.activation(out, in_, func, bias=bias, scale=scale)

            def store_act(dst, ps, bcol, n):
                """dst(fp8) = relu(psum + bias): single instruction."""
                if n % 2 == 0:
                    nc.vector.tensor_scalar(dst, ps, bcol, 0.0, AL.add, AL.max)
                else:
                    nc.scalar.activation(dst, ps, AF.Relu, bias=bcol)

            HPI = float(np.pi / 2)
            PI = float(np.pi)

            def epilogue_pre():
                """x-only QP/barrier quantities for ALL batch tiles at once
                ([32, NBT*GPB] ops); runs on Vector/Scalar while the PE is
                in the dense layers."""
                NF = NBT * GPB
                Xv = Xep.rearrange("p (f j) -> p f j", j=5)

                def T(nm):
                    return ep.tile([32, NF], F32, tag=nm, bufs=1,
                                   name=f"{nm}_pre")

                def emul(o, a, b):
                    nc.vector.tensor_mul(o, a, b)

                def eadd(o, a, b):
                    nc.vector.tensor_add(o, a, b)

                def stt(o, a, s, op0, b, op1):
                    nc.vector.scalar_tensor_tensor(o, a, float(s), b, op0, op1)

                t1r, w1r = Xv[:, :, 0], Xv[:, :, 1]
                t2r, w2r = Xv[:, :, 2], Xv[:, :, 3]

                if float(std[0]) == 1.0 and float(mean[0]) == 0.0:
                    t1m = t1r
                else:
                    t1m = T("t1m"); eact(t1m, t1r, AF.Copy, bias=float(mean[0]), scale=float(std[0]))
                if float(std[2]) == 1.0 and float(mean[2]) == 0.0:
                    t2m = t2r
                else:
                    t2m = T("t2m"); eact(t2m, t2r, AF.Copy, bias=float(mean[2]), scale=float(std[2]))

                def sincos(theta, nm):
                    ws = T(nm + "_ws"); nc.vector.add_range_wrap(ws, theta, 0.0, PI, 2 * PI)
                    s = T(nm + "_s"); eact(s, ws, AF.Sin)
                    wc = T(nm + "_wc"); nc.vector.add_range_wrap(wc, theta, HPI, PI, 2 * PI)
                    c = T(nm + "_c"); eact(c, wc, AF.Sin)
                    return s, c

                s1, c1 = sincos(t1m, "t1")
                s2, c2 = sincos(t2m, "t2")

                if float(std[1]) == 1.0 and float(mean[1]) == 0.0:
                    w1v_ = w1r
                else:
                    w1v_ = T("w1v"); eact(w1v_, w1r, AF.Copy, bias=float(mean[1]), scale=float(std[1]))
                if float(std[3]) == 1.0 and float(mean[3]) == 0.0:
                    w2v_ = w2r
                else:
                    w2v_ = T("w2v"); eact(w2v_, w2r, AF.Copy, bias=float(mean[3]), scale=float(std[3]))

                pxu = T("pxu"); eadd(pxu, c1, c2)
                px = T("px"); eact(px, pxu, AF.Copy, bias=-OBS_X, scale=L1C)
                pyu = T("pyu"); eadd(pyu, s1, s2)
                py = T("py"); eact(py, pyu, AF.Copy, bias=-OBS_Y, scale=L1C)

                a1 = T("a1"); emul(a1, s1, w1v_)
                a2 = T("a2"); emul(a2, s2, w2v_)
                vxn = T("vxn"); eadd(vxn, a1, a2)          # = -vx/3
                bb1 = T("bb1"); emul(bb1, c1, w1v_)
                bb2 = T("bb2"); emul(bb2, c2, w2v_)
                vyu = T("vyu"); eadd(vyu, bb1, bb2)
                vy = T("vy"); eact(vy, vyu, AF.Copy, scale=3.0)

                q1 = T("q1"); emul(q1, px, vxn)
                q2 = T("q2"); emul(q2, py, vy)
                bdot2 = T("bdot2"); stt(bdot2, q1, -3.0, AL.mult, q2, AL.add)

                w1sq = T("w1sq"); emul(w1sq, w1v_, w1v_)
                w2sq = T("w2sq"); emul(w2sq, w2v_, w2v_)
                cw1 = T("cw1"); emul(cw1, c1, w1sq)
                cw2 = T("cw2"); emul(cw2, c2, w2sq)
                cw = T("cw"); eadd(cw, cw1, cw2)
                sw1 = T("sw1"); emul(sw1, s1, w1sq)
                sw2 = T("sw2"); emul(sw2, s2, w2sq)
                sw = T("sw"); eadd(sw, sw1, sw2)
                t1x = T("t1x"); emul(t1x, px, cw)
                t2y = T("t2y"); emul(t2y, py, sw)
                txy = T("txy"); eadd(txy, t1x, t2y)
                vv1 = T("vv1"); emul(vv1, vxn, vxn)
                vv2 = T("vv2"); emul(vv2, vy, vy)
                vv = T("vv"); stt(vv, vv1, 9.0, AL.mult, vv2, AL.add)
                Lhalf = T("Lhalf"); stt(Lhalf, txy, -3.0, AL.mult, vv, AL.add)

                g1a = T("g1a"); emul(g1a, px, s1)
                g1b = T("g1b"); emul(g1b, py, c1)
                g2a = T("g2a"); emul(g2a, px, s2)
                g2b = T("g2b"); emul(g2b, py, c2)
                G12 = ep.tile([32, NF * 2], F32, tag="G12", bufs=1,
                              name="G12_pre")
                G12v = G12.rearrange("p (f q) -> p f q", q=2)
                G1h, G2h = G12v[:, :, 0], G12v[:, :, 1]
                stt(G1h, g1b, -1.0, AL.mult, g1a, AL.add)  # G1/6
                stt(G2h, g2b, -1.0, AL.mult, g2a, AL.add)  # G2/6

                pxsq = T("pxsq"); emul(pxsq, px, px)
                pysq = T("pysq"); emul(pysq, py, py)
                bar = T("bar"); stt(bar, pxsq, -RADIUS * RADIUS, AL.add, pysq, AL.add)

                d1 = T("d1"); emul(d1, G1h, G1h)
                d2 = T("d2"); emul(d2, G2h, G2h)
                den36 = T("den36"); stt(den36, d1, 1e-12 / 36.0, AL.add, d2, AL.add)
                nrec = T("nrec"); nc.vector.reciprocal(nrec, den36)

                return dict(bdot2=bdot2, bar=bar, Lhalf=Lhalf,
                            G1h=G1h, G2h=G2h, G12=G12, nrec=nrec)

            def epilogue_post(bt, vta, vtb, pre):
                """Head-dependent tail of the QP for batch tile bt."""
                Yva = vta.rearrange("p (f q) -> p f q", q=32)
                Yvb = vtb.rearrange("p (f q) -> p f q", q=32)
                sg1, sg2 = Yvb[:, :, 0], Yvb[:, :, 1]
                OUTv = OUT[:, bt * GPB * 2:(bt + 1) * GPB * 2] \
                    .rearrange("p (f i) -> p f i", i=2)
                fsl = slice(bt * GPB, (bt + 1) * GPB)

                def T(nm):
                    return ep.tile([32, GPB], F32, tag=nm, bufs=NBT,
                                   name=f"{nm}_b{bt}")

                def emul(o, a, b):
                    nc.vector.tensor_mul(o, a, b)

                def eadd(o, a, b):
                    nc.vector.tensor_add(o, a, b)

                def stt(o, a, s, op0, b, op1):
                    nc.vector.scalar_tensor_tensor(o, a, float(s), b, op0, op1)

                bdot2, bar = pre["bdot2"][:, fsl], pre["bar"][:, fsl]
                Lhalf, nrec = pre["Lhalf"][:, fsl], pre["nrec"][:, fsl]
                G12s = pre["G12"].rearrange("p (f q) -> p f q", q=2)[:, fsl, :]
                P12 = Yva[:, :, 0:2]  # [32, GPB, 2]

                ssum = T("ssum"); eadd(ssum, sg1, sg2)
                sprod = T("sprod"); emul(sprod, sg1, sg2)
                hb_ = T("hb_"); emul(hb_, ssum, bdot2)
                hc = T("hc"); emul(hc, sprod, bar)

                r12 = ep.tile([32, GPB * 2], F32, tag="r12", bufs=NBT,
                              name=f"r12_b{bt}")
                r12v = r12.rearrange("p (f q) -> p f q", q=2)
                nc.vector.tensor_mul(r12v, G12s, P12)
                rs = T("rs"); eadd(rs, r12v[:, :, 0], r12v[:, :, 1])
                va2 = T("va2"); stt(va2, hc, 8.0, AL.mult, Lhalf, AL.add)
                va = T("va"); stt(va, hb_, 4.0, AL.mult, va2, AL.add)  # h/2
                vb = T("vb"); stt(vb, rs, 3.0, AL.mult, va, AL.add)    # viol=-2vb

                vr = T("vr")
                nc.vector.tensor_scalar(vr, vb, -1.0, 0.0, AL.mult, AL.max)
                lam18 = T("lam18"); emul(lam18, vr, nrec)

                lam18b = bass.AP(tensor=lam18.tensor, offset=lam18.offset,
                                 ap=list(lam18.ap) + [[0, 2]])
                lg12 = ep.tile([32, GPB * 2], F32, tag="lg12", bufs=NBT,
                               name=f"lg12_b{bt}")
                lg12v = lg12.rearrange("p (f q) -> p f q", q=2)
                nc.vector.tensor_mul(lg12v, lam18b, G12s)
                if (float(sl[0]) == 1.0 and float(sl[1]) == 1.0
                        and float(ml[0]) == 0.0 and float(ml[1]) == 0.0):
                    # out = -(lg12/3 + P12): one DVE op straight into OUT
                    stt(OUTv[:, :, 0:2], lg12v, -1.0 / 3.0, AL.mult, P12,
                        AL.subtract)
                else:
                    u12n = ep.tile([32, GPB * 2], F32, tag="u12n", bufs=NBT,
                                   name=f"u12n_b{bt}")
                    u12v = u12n.rearrange("p (f q) -> p f q", q=2)
                    stt(u12v, lg12v, 1.0 / 3.0, AL.mult, P12, AL.add)
                    eact(OUTv[:, :, 0], u12v[:, :, 0], AF.Copy,
                         bias=-float(ml[0]) / float(sl[0]),
                         scale=-1.0 / float(sl[0]))
                    eact(OUTv[:, :, 1], u12v[:, :, 1], AF.Copy,
                         bias=-float(ml[1]) / float(sl[1]),
                         scale=-1.0 / float(sl[1]))

            def pair_tiles(nm, n_pairs, bt):
                return [ap_.tile([128, 2 * BT], FP8, tag="act",
                                 name=f"{nm}_p{t}b{bt}")
                        for t in range(n_pairs)]

            def layer1(bt):
                """L1 (f32r, K=5) for one batch tile -> fp8 pair tiles."""
                x1p = pair_tiles("x1", N1 // 2, bt)
                rhs1 = xT[:, bt * BT:(bt + 1) * BT]
                for n in range(N1):
                    ps = pmm.tile([128, BT], F32, tag="pm", name=f"ps1_{n}b{bt}")
                    nc.tensor.matmul(ps, w1[:, n * 128:(n + 1) * 128], rhs1,
                                     start=True, stop=True)
                    store_act(x1p[n // 2][:, (n % 2) * BT:(n % 2 + 1) * BT],
                              ps, biasp[:, BOF["l1"] + n:BOF["l1"] + n + 1], n)
                return x1p

            def dense_dr(nm, in_pairs, wv, n_pairs_k, n_out, bof, bt):
                """fp8 DoubleRow dense layer -> fp8 pair tiles."""
                outp = pair_tiles(nm, n_out // 2, bt)
                for n in range(n_out):
                    ps = pmm.tile([128, BT], F32, tag="pm",
                                  name=f"ps{nm}_{n}b{bt}")
                    for t in range(n_pairs_k):
                        rhs = in_pairs[t].rearrange("p (i b) -> p i b", i=2)
                        nc.tensor.matmul(
                            ps, wv[:, t, :, n * 128:(n + 1) * 128], rhs,
                            start=(t == 0), stop=(t == n_pairs_k - 1),
                            perf_mode=DR)
                    store_act(outp[n // 2][:, (n % 2) * BT:(n % 2 + 1) * BT],
                              ps, biasp[:, bof + n:bof + n + 1], n)
                return outp

            def batch_tile_pipeline(bt, x1p, pre):
                """Layers 2+ for one 512-sample batch tile."""
                sfx = f"b{bt}"
                x2p = dense_dr("x2", x1p, w2v, KP2, N2, BOF["l2"], bt)
                x31p = dense_dr("x31", x2p, w31v, KP3, N3, BOF["l31"], bt)
                x32p = dense_dr("x32", x2p, w32v, KP3, N3, BOF["l32"], bt)
                x41p = dense_dr("x41", x31p, w41v, KP4, N4, BOF["l41"], bt)
                x42p = dense_dr("x42", x32p, w42v, KP4, N4, BOF["l42"], bt)

                # heads: DoubleRow into [32, BT] psum (rows 0:2 valid).
                # x5a/x5b rows 2:31 stay uninitialized — the transpose puts
                # them in columns epilogue_post never reads.
                x5a = mp.tile([32, BT], F32, tag="x5a", bufs=2, name=f"x5a{sfx}")
                x5b = mp.tile([32, BT], F32, tag="x5b", bufs=2, name=f"x5b{sfx}")
                ph1 = phd.tile([32, BT], F32, tag="ph", name=f"ph1{sfx}")
                for t in range(KP5):
                    rhs = x41p[t].rearrange("p (i b) -> p i b", i=2)
                    nc.tensor.matmul(ph1, w5v[:, 0, t, :, :], rhs,
                                     start=(t == 0), stop=(t == KP5 - 1),
                                     perf_mode=DR)
                nc.scalar.activation(x5a[0:2, :], ph1[0:2, :], AF.Identity,
                                     bias=hb[:, 0:1], scale=hb[:, 2:3])
                ph2 = phd.tile([32, BT], F32, tag="ph", name=f"ph2{sfx}")
                for t in range(KP5):
                    rhs = x42p[t].rearrange("p (i b) -> p i b", i=2)
                    nc.tensor.matmul(ph2, w5v[:, 1, t, :, :], rhs,
                                     start=(t == 0), stop=(t == KP5 - 1),
                                     perf_mode=DR)
                nc.scalar.activation(x5b[0:2, :], ph2[0:2, :], AF.Sigmoid,
                                     bias=hb[:, 1:2], scale=hb[:, 3:4])

                # DVE 32x32 stream transpose: sample j=32b+c -> [c, 32b+row]
                vta = mp.tile([32, BT], F32, tag="vta", bufs=2, name=f"vta{sfx}")
                vtb = mp.tile([32, BT], F32, tag="vtb", bufs=2, name=f"vtb{sfx}")
                nc.vector.transpose(vta, x5a)
                nc.vector.transpose(vtb, x5b)

                epilogue_post(bt, vta, vtb, pre)
                nc.sync.dma_start(
                    out=out_d[:, bt * GPB * 2:(bt + 1) * GPB * 2],
                    in_=OUT[:, bt * GPB * 2:(bt + 1) * GPB * 2])

            x1_all = [layer1(bt) for bt in range(NBT)]
            pre = epilogue_pre()
            for bt in range(NBT):
                batch_tile_pipeline(bt, x1_all[bt], pre)

    nc.compile()
    return nc


def _q8(a, scale):
    import ml_dtypes
    v = np.clip(np.asarray(a, np.float64) * scale, -240.0, 240.0)
    return v.astype(ml_dtypes.float8_e4m3)


def _pack_pairs(Wq, K, N):
    """[K, N] fp8 -> [128, (K/256)*2*N] with [p, t, i, n] = W[(2t+i)*128+p, n]."""
    return np.ascontiguousarray(
        Wq.reshape(K // 256, 2, 128, N).transpose(2, 0, 1, 3)
        .reshape(128, (K // 256) * 2 * N))


def prep_inputs(x, W1, b1, W2, b2, W31, b31, W32, b32,
                W41, b41, W42, b42, W51, b51, W52, b52):
    """Host-side calibration, quantization, packing -> per-core in_maps."""
    f32 = np.float32
    x = np.asarray(x, f32)
    Ws = {k: np.asarray(v, f32) for k, v in
          dict(W1=W1, W2=W2, W31=W31, W32=W32, W41=W41, W42=W42,
               W51=W51, W52=W52).items()}
    bs = {k: np.asarray(v, f32) for k, v in
          dict(b1=b1, b2=b2, b31=b31, b32=b32, b41=b41, b42=b42,
               b51=b51, b52=b52).items()}

    # calibration forward (fp32) for activation absmax
    relu = lambda v: np.maximum(v, 0.0)
    c1 = relu(x @ Ws["W1"] + bs["b1"])
    c2 = relu(c1 @ Ws["W2"] + bs["b2"])
    c31 = relu(c2 @ Ws["W31"] + bs["b31"])
    c32 = relu(c2 @ Ws["W32"] + bs["b32"])
    c41 = relu(c31 @ Ws["W41"] + bs["b41"])
    c42 = relu(c32 @ Ws["W42"] + bs["b42"])
    amax = {k: max(float(np.abs(v).max()), 1e-6) for k, v in
            dict(x1=c1, x2=c2, x31=c31, x32=c32, x41=c41, x42=c42).items()}
    del c1, c2, c31, c32, c41, c42

    a1 = MARGIN / amax["x1"]

    def beta_for(a_in, amax_out):
        return 2.0 ** np.floor(np.log2((MARGIN / amax_out) / a_in))

    b2s = beta_for(a1, amax["x2"]);      a2 = b2s * a1
    b31s = beta_for(a2, amax["x31"]);    a31 = b31s * a2
    b32s = beta_for(a2, amax["x32"]);    a32 = b32s * a2
    b41s = beta_for(a31, amax["x41"]);   a41 = b41s * a31
    b42s = beta_for(a32, amax["x42"]);   a42 = b42s * a32
    b51s = 192.0 / max(float(np.abs(Ws["W51"]).max()), 1e-6)
    b52s = 192.0 / max(float(np.abs(Ws["W52"]).max()), 1e-6)

    # packed biases [128, 32]: per layer, alpha_out * b reshaped (chunks, 128).T
    bias_pack = np.zeros((128, 32), f32)
    for key, bvec, a_out, nch in [
            ("l1", bs["b1"], a1, N1), ("l2", bs["b2"], a2, N2),
            ("l31", bs["b31"], a31, N3), ("l32", bs["b32"], a32, N3),
            ("l41", bs["b41"], a41, N4), ("l42", bs["b42"], a42, N4)]:
        col = BOF[key]
        bias_pack[:, col:col + nch] = (a_out * bvec).reshape(nch, 128).T

    hbp = np.zeros((2, 4), f32)
    hbp[:, 0] = bs["b51"]
    hbp[:, 1] = bs["b52"]
    hbp[:, 2] = 1.0 / (b51s * a41)
    hbp[:, 3] = 1.0 / (b52s * a42)

    # head weights: pad N 2->32, quantize, pack; concat heads
    def head_pack(Wn, beta):
        Wq = np.zeros((D4, 32), np.float64)
        Wq[:, 0:2] = np.asarray(Wn, np.float64) * beta
        return _pack_pairs(_q8(Wq, 1.0), D4, 32)

    import ml_dtypes
    w5p = np.concatenate(
        [head_pack(Ws["W51"], b51s), head_pack(Ws["W52"], b52s)], axis=1)

    shared = {
        "W1s": np.ascontiguousarray(a1 * Ws["W1"]),
        "W2p": _pack_pairs(_q8(Ws["W2"], b2s), D1, D2),
        "W31p": _pack_pairs(_q8(Ws["W31"], b31s), D2, D3),
        "W32p": _pack_pairs(_q8(Ws["W32"], b32s), D2, D3),
        "W41p": _pack_pairs(_q8(Ws["W41"], b41s), D3, D4),
        "W42p": _pack_pairs(_q8(Ws["W42"], b42s), D3, D4),
        "W5p": np.ascontiguousarray(w5p),
        "biasp": bias_pack,
        "hbp": hbp,
    }
    in_maps = []
    for c in range(N_CORES):
        xc = x[c * BC:(c + 1) * BC]
        m = dict(shared)
        m["xT"] = np.ascontiguousarray(xc.T)
        m["Xep"] = np.ascontiguousarray(
            xc.reshape(BC // 32, 32, 5).transpose(1, 0, 2)
            .reshape(32, (BC // 32) * 5))
        in_maps.append(m)
    return in_maps


def unpack_output(results):
    outs = []
    for c in range(N_CORES):
        o = results[c]["out"]  # [32, (BC//32)*2]
        outs.append(o.reshape(32, BC // 32, 2).transpose(1, 0, 2).reshape(BC, 2))
    return np.ascontiguousarray(np.concatenate(outs, axis=0), dtype=np.float32)


_PROG_CACHE = {}


def get_program(consts_key):
    if consts_key not in _PROG_CACHE:
        _PROG_CACHE[consts_key] = build_program(consts_key)
    return _PROG_CACHE[consts_key]


def kernel(x, sgn, mean, std, mean_label, std_label,
           W1, b1, W2, b2, W31, b31, W32, b32,
           W41, b41, W42, b42, W51, b51, W52, b52,
           _trace=False, _tmpdir=None):
    assert int(np.asarray(sgn)) == 1
    consts = (
        tuple(float(v) for v in np.asarray(mean, np.float32)),
        tuple(float(v) for v in np.asarray(std, np.float32)),
        tuple(float(v) for v in np.asarray(mean_label, np.float32)),
        tuple(float(v) for v in np.asarray(std_label, np.float32)),
    )
    nc = get_program(consts)
    in_maps = prep_inputs(x, W1, b1, W2, b2, W31, b31, W32, b32,
                          W41, b41, W42, b42, W51, b51, W52, b52)
    res = run_bass_kernel_spmd(nc, in_maps, core_ids=list(range(N_CORES)),
                               trace=_trace, tmpdir=_tmpdir)
    out = unpack_output(res.results)
    kernel.last_result = res
    return out
